# revision 26
# baseline (speedup 1.0000x reference)
"""Distributed causal multi-head attention block (GPT-2 style) for 8 TRN2 NeuronCores.

Sharding: data-parallel over batch (4 pairs of cores) x tensor-parallel over
heads (2 groups of 8 heads). Core c handles batch c//2, head-group c%2.

v4 reorganization vs v3:
  - ~33 junk matmuls on memset SBUF data at t=0 warm the PE/HAM clock gate
    through the ~9us DMA-bootstrap dead time, so real matmuls run at 2.4 GHz.
  - A tiny warmup AllReduce flushes the collective-stream barrier early.
  - Attention runs per head-PAIR: the two heads of a feature tile (partition
    rows 0:64 / 64:128) issue score matmuls back-to-back at row groups (0,0)
    and (64,0), so both K=64 matmuls stream concurrently through the PE.
  - Causal masking is folded into the scores PSUM via an identity-matmul that
    accumulates -240 above the diagonal BEFORE exp (exp then yields ~0), so
    the DVE is out of the scores->exp->PV critical chain.
  - One ACT exp per (pair, k-block): strided [128, 2, N] over both heads.
  - Head-major schedule: each (pair, half) finishes and immediately stages its
    aT rows for a pair-AllReduce(add); the partner's rows are recovered
    rank-agnostically as (sum - mine) on GpSimd. c_proj contracts own rows
    straight from SBUF and partner rows from the recovered copies, with
    host-reordered Wproj rows [own 512 | partner 512]. Only the last 128KB
    exchange sits near the tail, and its chunk-3 contribution uses the
    (sum*W + mine*(-W)) trick to skip the subtract latency.
  - c_proj per out-tile runs in two PSUM sessions (bias+chunks 0,1 early,
    chunks 2,3 late) joined by an SBUF f32 partial, so early proj matmuls
    interleave with late attention pairs on a 2-bank rotation.
"""

import numpy as np
import ml_dtypes

import concourse.bass as bass
import concourse.mybir as mybir
import concourse.tile as tile
from concourse import bacc
from concourse.bass_utils import run_bass_kernel_spmd
from concourse.masks import make_identity, make_lower_triangular

F32 = mybir.dt.float32
BF16 = mybir.dt.bfloat16
AF = mybir.ActivationFunctionType
ALU = mybir.AluOpType

P = 128
S = 1024          # sequence length
NX = 1024         # model width
D = 64            # head dim
H_LOC = 8         # heads per core
FEAT = H_LOC * D  # 512 local attention features
NKC = NX // P     # 8 contraction chunks
NST = S // P      # 8 sequence tiles
VW = D + 1        # v block width incl. ones column (65)
N_JUNK = 8        # warmup matmuls to open the HAM clock gate early

PAIRS = [[0, 1], [2, 3], [4, 5], [6, 7]]


def build():
    nc = bacc.Bacc(num_devices=8)
    xT = nc.dram_tensor("xT", [NX, S], BF16, kind="ExternalInput")
    wqkv = nc.dram_tensor("wqkv", [NX, 3 * FEAT], BF16, kind="ExternalInput")
    bqk_t = nc.dram_tensor("bqk_t", [P, 8], F32, kind="ExternalInput")
    bv_r = nc.dram_tensor("bv_r", [1, FEAT], BF16, kind="ExternalInput")
    wproj = nc.dram_tensor("wproj", [NX, FEAT], BF16, kind="ExternalInput")
    wpn3 = nc.dram_tensor("wpn3", [P, FEAT], BF16, kind="ExternalInput")
    bp_r = nc.dram_tensor("bp_r", [1, FEAT], BF16, kind="ExternalInput")
    out = nc.dram_tensor("out", [S, FEAT], F32, kind="ExternalOutput")

    with tile.TileContext(nc) as tc:
        with (
            tc.tile_pool(name="pt", bufs=8) as ptp,            # P^T pair blocks
            tc.tile_pool(name="small", bufs=2) as small,       # recip vectors
            tc.tile_pool(name="outp", bufs=3) as outp,         # out f32 tiles
            tc.tile_pool(name="dram", bufs=1, space="DRAM") as dram,
            tc.tile_pool(name="resident", bufs=1) as res,
        ):
            # ---- resident SBUF tensors ----
            xT_all = res.tile([P, NKC * S], BF16, tag="xT_all")
            wqkv_sb = res.tile([P, NKC * 3 * FEAT], BF16, tag="wqkv_sb")
            qkT_all = res.tile([P, 8 * S], BF16, tag="qkT_all")   # qT(0..3)|kT(4..7)
            v_sb = res.tile([P, NST * H_LOC * VW], BF16, tag="v_sb")
            aT_loc = res.tile([P, 4 * S], BF16, tag="aT_loc")     # my 512 feats
            blk0_sb = res.tile([P, 4 * S], BF16, tag="blk0_sb")   # gathered rank0
            blk1_sb = res.tile([P, 4 * S], BF16, tag="blk1_sb")   # gathered rank1
            part_sb = res.tile([P, 4 * S], BF16, tag="part_sb")   # partner rows
            wp_sb = res.tile([P, NKC * FEAT], BF16, tag="wp_sb")  # [own|partner]
            wpn3_sb = res.tile([P, FEAT], BF16, tag="wpn3_sb")    # -W partner ch3
            partial_sb = res.tile([P, 4 * FEAT], F32, tag="partial_sb")
            bias_sb = res.tile([P, 8], F32, tag="bias_sb")
            bv_row = res.tile([1, FEAT], BF16, tag="bv_row")
            bp_row = res.tile([1, FEAT], BF16, tag="bp_row")
            ones_row = res.tile([1, P], BF16, tag="ones_row")
            junk_sb = res.tile([P, 512], BF16, tag="junk_sb")
            iden = res.tile([P, P], BF16, tag="iden")
            mask_add = res.tile([P, P], BF16, tag="mask_add")

            nc.vector.memset(ones_row[:], 1.0)
            nc.vector.memset(junk_sb[:], 0.001)
            nc.vector.memset(v_sb[:], 1.0)
            make_identity(nc, iden[:])
            make_lower_triangular(nc, mask_add[:], val=-240.0, diag=False)

            # ---- input stream. sync queue: x/w chunks (critical path) first,
            # then wproj + wpn3, then collective-sum reloads, then late out
            # tiles. gpsimd queue: biases, warmup-cc staging, per-half
            # broadcasts / staging / subs, early out tiles. ----
            for kc in range(NKC):
                xs = slice(kc * P, (kc + 1) * P)
                if kc < 2:
                    nc.sync.dma_start(
                        wqkv_sb[:, kc * 3 * FEAT : kc * 3 * FEAT + 640],
                        wqkv[xs, 0:640],
                    )
                    nc.sync.dma_start(
                        xT_all[:, kc * S : kc * S + 512], xT[xs, 0:512]
                    )
                    nc.sync.dma_start(
                        xT_all[:, kc * S + 512 : (kc + 1) * S], xT[xs, 512:1024]
                    )
                    nc.sync.dma_start(
                        wqkv_sb[:, kc * 3 * FEAT + 640 : (kc + 1) * 3 * FEAT],
                        wqkv[xs, 640:1536],
                    )
                else:
                    nc.sync.dma_start(xT_all[:, kc * S : (kc + 1) * S], xT[xs, :])
                    nc.sync.dma_start(
                        wqkv_sb[:, kc * 3 * FEAT : (kc + 1) * 3 * FEAT],
                        wqkv[xs, :],
                    )
            # warmup collective first on the gpsimd queue: DRAM->DRAM staging
            # with no compute deps, so the cc-stream barrier runs during boot.
            # The same tiny op is re-issued as a pair-resync right before the
            # big gathers, so partner skew is absorbed by a 32-byte op instead
            # of the data-carrying one.
            cc_w_in = dram.tile([1, 8], F32, name="cc_w_in")
            cc_w_outs = [dram.tile([2, 8], F32, name=f"cc_w_out{i}") for i in range(3)]
            nc.gpsimd.dma_start(cc_w_in[:, :], bqk_t[0:1, 0:8])

            def cc_resync(i):
                nc.gpsimd.collective_compute(
                    "AllGather", ALU.bypass, replica_groups=PAIRS,
                    ins=[cc_w_in[:].opt()], outs=[cc_w_outs[i][:].opt()],
                )

            cc_resync(0)
            nc.gpsimd.dma_start(bias_sb[:], bqk_t[:, :])
            nc.gpsimd.dma_start(bv_row[:], bv_r[:, :])
            nc.gpsimd.dma_start(bp_row[:], bp_r[:, :])
            for fc in range(NKC):
                nc.sync.dma_start(
                    wp_sb[:, fc * FEAT : (fc + 1) * FEAT],
                    wproj[fc * P : (fc + 1) * P, :],
                )
            nc.sync.dma_start(wpn3_sb[:], wpn3[:, :])

            # ---- collective staging: one big qh0 gather, 3 qh1 parts ----
            cc_inA = dram.tile([4 * P, 512], BF16, name="cc_inA")
            cc_outA = dram.tile([8 * P, 512], BF16, name="cc_outA")
            B_PARTS = [[0], [1], [2], [3]]
            cc_inB = [
                dram.tile([len(ps_) * P, 512], BF16, name=f"cc_inB{i}")
                for i, ps_ in enumerate(B_PARTS)
            ]
            cc_outB = [
                dram.tile([2 * len(ps_) * P, 512], BF16, name=f"cc_outB{i}")
                for i, ps_ in enumerate(B_PARTS)
            ]

            # ---- qkv group helpers ----
            def qk_mm(ps, ft, half, kc):
                nc.tensor.matmul(
                    ps[:],
                    wqkv_sb[:, kc * 3 * FEAT + ft * P : kc * 3 * FEAT + (ft + 1) * P],
                    xT_all[:, kc * S + half * 512 : kc * S + (half + 1) * 512],
                    start=(kc == 0),
                    stop=(kc == NKC - 1),
                )

            def qk_consume(ps, ft, half):
                nc.vector.tensor_scalar_add(
                    out=qkT_all[:, ft * S + half * 512 : ft * S + (half + 1) * 512],
                    in0=ps[:],
                    scalar1=bias_sb[:, ft : ft + 1],
                )

            def v_bias(ps):
                nc.tensor.matmul(ps[:], ones_row[:, 0:P], bv_row[:], start=True, stop=False)

            def v_mm(ps, st, kc):
                nc.tensor.matmul(
                    ps[:],
                    xT_all[:, kc * S + st * P : kc * S + (st + 1) * P],
                    wqkv_sb[:, kc * 3 * FEAT + 1024 : kc * 3 * FEAT + 1536],
                    start=False,
                    stop=(kc == NKC - 1),
                )

            def v_consume(ps, st):
                base = st * H_LOC * VW
                dst = v_sb[:, base : base + H_LOC * VW].rearrange(
                    "p (h w) -> p h w", h=H_LOC
                )[:, :, 0:D]
                src = ps[:].rearrange("p (h d) -> p h d", h=H_LOC)
                nc.vector.tensor_copy(out=dst, in_=src)

            # ---- shared PSUM pools for wave1 + attention + proj (no barrier) ----
            with (
                tc.tile_pool(name="ps_sc", bufs=2, space="PSUM") as ps_sc,
                tc.tile_pool(name="ps_pa", bufs=2, space="PSUM") as ps_pa,
                tc.tile_pool(name="ps_sm", bufs=2, space="PSUM") as ps_sm,
            ):
                # wave 1: junk warmup + 6 groups fed in DMA-arrival order,
                # allocated from the shared pools so attention needs no
                # pool-close barrier to start
                junk_ps = ps_sm.tile([P, 512], F32, name="junk_ps", tag="sm")
                for _ in range(N_JUNK):
                    nc.tensor.matmul(
                        junk_ps[:], junk_sb[:, 0:P], junk_sb[:, 0:512],
                        start=True, stop=True,
                    )
                w1ps = {
                    ("qk", 0, 0): ps_sc.tile([P, 1024], F32, name="w1q00", tag="sc"),
                    ("qk", 0, 1): ps_sc.tile([P, 1024], F32, name="w1q01", tag="sc"),
                    ("qk", 4, 0): ps_pa.tile([P, 512], F32, name="w1k40", tag="pa"),
                    ("qk", 4, 1): ps_pa.tile([P, 512], F32, name="w1k41", tag="pa"),
                }
                for st in (0, 1):
                    ps = ps_sm.tile([P, 512], F32, name=f"w1v{st}", tag="sm")
                    w1ps[("v", st)] = ps
                    v_bias(ps)
                for kc in range(NKC):
                    for ft, half in [(0, 0), (4, 0), (0, 1), (4, 1)]:
                        qk_mm(w1ps[("qk", ft, half)][:, 0:512], ft, half, kc)
                    for st in (0, 1):
                        v_mm(w1ps[("v", st)], st, kc)
                for ft, half in [(0, 0), (0, 1), (4, 0), (4, 1)]:
                    qk_consume(w1ps[("qk", ft, half)][:, 0:512], ft, half)
                for st in (0, 1):
                    v_consume(w1ps[("v", st)], st)
                def qkT_tile(ft):
                    for half in range(2):
                        ps = ps_sm.tile([P, 512], F32, name="ps_qk", tag="sm")
                        for kc in range(NKC):
                            qk_mm(ps, ft, half, kc)
                        qk_consume(ps, ft, half)

                def v_tile(st):
                    ps = ps_sm.tile([P, 512], F32, name="ps_v", tag="sm")
                    v_bias(ps)
                    for kc in range(NKC):
                        v_mm(ps, st, kc)
                    v_consume(ps, st)

                def recover(p, qh):
                    # partner chunk p half qh = (block0 + block1) - my rows, DVE
                    col = p * S + qh * 512
                    nc.vector.tensor_tensor(
                        out=part_sb[:, col : col + 512],
                        in0=blk0_sb[:, col : col + 512],
                        in1=blk1_sb[:, col : col + 512],
                        op=ALU.add,
                    )
                    nc.vector.tensor_tensor(
                        out=part_sb[:, col : col + 512],
                        in0=part_sb[:, col : col + 512],
                        in1=aT_loc[:, col : col + 512],
                        op=ALU.subtract,
                    )

                def stage_A():
                    # all four chunks' qh0 halves -> 512KB pair AllGather
                    for p in range(4):
                        nc.gpsimd.dma_start(
                            cc_inA[p * P : (p + 1) * P, :],
                            aT_loc[:, p * S : p * S + 512],
                        )
                    nc.gpsimd.collective_compute(
                        "AllGather", ALU.bypass, replica_groups=PAIRS,
                        ins=[cc_inA[:].opt()], outs=[cc_outA[:].opt()],
                    )

                def reload_A():
                    for p in range(4):
                        nc.sync.dma_start(
                            blk0_sb[:, p * S : p * S + 512],
                            cc_outA[p * P : (p + 1) * P, :],
                        )
                        nc.sync.dma_start(
                            blk1_sb[:, p * S : p * S + 512],
                            cc_outA[(4 + p) * P : (5 + p) * P, :],
                        )

                def stage_B(i):
                    ps_ = B_PARTS[i]
                    for k, p in enumerate(ps_):
                        nc.gpsimd.dma_start(
                            cc_inB[i][k * P : (k + 1) * P, :],
                            aT_loc[:, p * S + 512 : (p + 1) * S],
                        )
                    nc.gpsimd.collective_compute(
                        "AllGather", ALU.bypass, replica_groups=PAIRS,
                        ins=[cc_inB[i][:].opt()], outs=[cc_outB[i][:].opt()],
                    )

                def reload_B(i):
                    ps_ = B_PARTS[i]
                    n = len(ps_)
                    for k, p in enumerate(ps_):
                        nc.sync.dma_start(
                            blk0_sb[:, p * S + 512 : (p + 1) * S],
                            cc_outB[i][k * P : (k + 1) * P, :],
                        )
                        nc.sync.dma_start(
                            blk1_sb[:, p * S + 512 : (p + 1) * S],
                            cc_outB[i][(n + k) * P : (n + k + 1) * P, :],
                        )

                def attn_half(pair, qh, filler=None):
                    nj = 4 * qh + 4
                    kcol = (4 + pair) * S
                    qbase = pair * S + qh * 512
                    pt_blocks = []
                    for j in range(nj):
                        dloc = j - 4 * qh
                        coff = max(dloc, 0) * P
                        diag = dloc >= 0
                        ps = ps_sc.tile([P, 1024], F32, name="ps_s", tag="sc")
                        ptb = ptp.tile([P, 1024], BF16, name="ptb", tag="pt")
                        for hh in range(2):
                            nc.tensor.matmul(
                                ps[:, hh * 512 + coff : hh * 512 + 512],
                                qkT_all[hh * D : (hh + 1) * D,
                                        kcol + j * P : kcol + (j + 1) * P],
                                qkT_all[hh * D : (hh + 1) * D,
                                        qbase + coff : qbase + 512],
                                start=True,
                                stop=not diag,
                            )
                        if diag:
                            for hh in range(2):
                                nc.tensor.matmul(
                                    ps[:, hh * 512 + coff : hh * 512 + coff + P],
                                    iden[:, 0:P],
                                    mask_add[:, 0:P],
                                    start=False,
                                    stop=True,
                                    skip_group_check=True,
                                )
                        pr = ps[:].rearrange("p (b n) -> p b n", b=2)[:, :, coff:512]
                        tr = ptb[:].rearrange("p (b n) -> p b n", b=2)[:, :, coff:512]
                        nc.scalar.activation(out=tr, in_=pr, func=AF.Exp, scale=0.125)
                        pt_blocks.append((ptb, coff))
                    # filler PE work runs here, while ACT chews the exps
                    if filler is not None:
                        filler()
                    psas = [
                        ps_pa.tile([P, 512], F32, name=f"psa{hh}", tag="pa")
                        for hh in range(2)
                    ]
                    for j, (ptb, coff) in enumerate(pt_blocks):
                        for hh in range(2):
                            h = 2 * pair + hh
                            nc.tensor.matmul(
                                psas[hh][:VW, coff:512],
                                v_sb[:, j * H_LOC * VW + h * VW
                                     : j * H_LOC * VW + (h + 1) * VW],
                                ptb[:, hh * 512 + coff : hh * 512 + 512],
                                start=(j == 0),
                                stop=(j == nj - 1),
                            )
                    # normalize per head: recip of denom row, broadcast, scale
                    acol = pair * S + qh * 512
                    for hh in range(2):
                        psa = psas[hh]
                        db = small.tile([1, 512], F32, tag="db")
                        nc.vector.tensor_copy(out=db[:], in_=psa[D : D + 1, 0:512])
                        rc = small.tile([1, 512], F32, tag="rc")
                        nc.vector.reciprocal_approx_fast(rc[:], db[:])
                        bcs = small.tile([D, 512], F32, tag="bcs")
                        nc.gpsimd.partition_broadcast(bcs[:], rc[:])
                        nc.vector.tensor_tensor(
                            out=aT_loc[hh * D : (hh + 1) * D, acol : acol + 512],
                            in0=bcs[:],
                            in1=psa[0:D, 0:512],
                            op=ALU.mult,
                        )


                # ---- c_proj helpers ----
                def proj_mm(ps, lhs_sb, col, wslice, start, stop):
                    nc.tensor.matmul(
                        ps[:], lhs_sb[:, col : col + P], wslice,
                        start=start, stop=stop,
                    )

                def proj_full(t):
                    # out rows t*128 (qh0): one session over all 8 chunks
                    ps = ps_sm.tile([P, 512], F32, name="ps_pf", tag="sm")
                    nc.tensor.matmul(
                        ps[:], ones_row[:, 0:P], bp_row[:], start=True, stop=False
                    )
                    for p in range(4):
                        proj_mm(ps, aT_loc, p * S + t * P,
                                wp_sb[:, p * FEAT : (p + 1) * FEAT], False, False)
                        proj_mm(ps, part_sb, p * S + t * P,
                                wp_sb[:, (4 + p) * FEAT : (5 + p) * FEAT],
                                False, p == 3)
                    ot = outp.tile([P, FEAT], F32, tag="ot")
                    nc.vector.tensor_copy(out=ot[:], in_=ps[:])
                    nc.gpsimd.dma_start(out[t * P : (t + 1) * P, :], ot[:])

                def projA2(t):
                    # out rows t*128 (qh1), session 1: bias + chunks 0,1 + own 2
                    ps = ps_sm.tile([P, 512], F32, name="ps_pA", tag="sm")
                    nc.tensor.matmul(
                        ps[:], ones_row[:, 0:P], bp_row[:], start=True, stop=False
                    )
                    for p in range(2):
                        proj_mm(ps, aT_loc, p * S + t * P,
                                wp_sb[:, p * FEAT : (p + 1) * FEAT], False, False)
                        proj_mm(ps, part_sb, p * S + t * P,
                                wp_sb[:, (4 + p) * FEAT : (5 + p) * FEAT],
                                False, False)
                    proj_mm(ps, aT_loc, 2 * S + t * P,
                            wp_sb[:, 2 * FEAT : 3 * FEAT], False, True)
                    nc.vector.tensor_copy(
                        out=partial_sb[:, (t - 4) * FEAT : (t - 3) * FEAT], in_=ps[:]
                    )

                def projB2(t):
                    # session 2: partner ch2, chunk 3 via (b0+b1)*W + mine*(-W)
                    ps = ps_sm.tile([P, 512], F32, name="ps_pB", tag="sm")
                    proj_mm(ps, part_sb, 2 * S + t * P,
                            wp_sb[:, 6 * FEAT : 7 * FEAT], True, False)
                    proj_mm(ps, aT_loc, 3 * S + t * P,
                            wp_sb[:, 3 * FEAT : 4 * FEAT], False, False)
                    proj_mm(ps, blk0_sb, 3 * S + t * P,
                            wp_sb[:, 7 * FEAT : 8 * FEAT], False, False)
                    proj_mm(ps, blk1_sb, 3 * S + t * P,
                            wp_sb[:, 7 * FEAT : 8 * FEAT], False, False)
                    proj_mm(ps, aT_loc, 3 * S + t * P, wpn3_sb[:, :], False, True)
                    ot = outp.tile([P, FEAT], F32, tag="ot")
                    nc.vector.tensor_tensor(
                        out=ot[:], in0=ps[:],
                        in1=partial_sb[:, (t - 4) * FEAT : (t - 3) * FEAT],
                        op=ALU.add,
                    )
                    nc.sync.dma_start(out[t * P : (t + 1) * P, :], ot[:])

                # ---- schedule: qh0 sweep, big gather, qh1 sweep + proj ----
                attn_half(0, 0, filler=lambda: (v_tile(2), v_tile(3),
                                                qkT_tile(1), qkT_tile(5)))
                attn_half(1, 0, filler=lambda: (qkT_tile(2), qkT_tile(6)))
                attn_half(2, 0, filler=lambda: (qkT_tile(3), qkT_tile(7)))
                attn_half(3, 0, filler=lambda: (v_tile(4), v_tile(5)))
                cc_resync(1)
                stage_A()
                reload_A()
                attn_half(0, 1, filler=lambda: (v_tile(6), v_tile(7)))
                stage_B(0)
                for p in range(4):
                    recover(p, 0)
                attn_half(1, 1, filler=lambda: proj_full(0))
                stage_B(1)
                reload_B(0)
                attn_half(2, 1, filler=lambda: (proj_full(1), proj_full(2)))
                stage_B(2)
                reload_B(1)
                recover(0, 1)
                recover(1, 1)
                cc_resync(2)
                attn_half(3, 1, filler=lambda: (proj_full(3), projA2(4), projA2(5)))
                stage_B(3)
                reload_B(2)
                recover(2, 1)
                reload_B(3)
                projA2(6)
                projA2(7)
                for t in range(4, 8):
                    projB2(t)

    nc.finalize()
    return nc


_NC_CACHE = None
_LAST_IN_MAPS = None


def kernel(x, c_attn_w, c_attn_b, c_proj_w, c_proj_b):
    global _NC_CACHE, _LAST_IN_MAPS
    x = np.asarray(x, dtype=np.float32)
    c_attn_w = np.asarray(c_attn_w, dtype=np.float32)
    c_attn_b = np.asarray(c_attn_b, dtype=np.float32)
    c_proj_w = np.asarray(c_proj_w, dtype=np.float32)
    c_proj_b = np.asarray(c_proj_b, dtype=np.float32)
    B = x.shape[0]
    assert x.shape == (B, S, NX)
    bf16 = ml_dtypes.bfloat16

    xTs = [np.ascontiguousarray(x[b].T).astype(bf16) for b in range(B)]
    in_maps = []
    for c in range(8):
        b, hg = c // 2, c % 2
        cols = slice(hg * FEAT, (hg + 1) * FEAT)
        wq = c_attn_w[:, 0 * NX :][:, cols]
        wk = c_attn_w[:, 1 * NX :][:, cols]
        bq = c_attn_b[0 * NX :][cols]
        bk = c_attn_b[1 * NX :][cols]
        bqk = np.concatenate([bq, bk])
        own = slice(hg * FEAT, (hg + 1) * FEAT)
        par = slice((1 - hg) * FEAT, (2 - hg) * FEAT)
        wproj_r = np.concatenate([c_proj_w[own, cols], c_proj_w[par, cols]], axis=0)
        wpn3 = -c_proj_w[par, cols][3 * P : 4 * P, :]
        in_maps.append(
            {
                "xT": xTs[b],
                "wqkv": np.ascontiguousarray(
                    np.concatenate([wq, wk, c_attn_w[:, 2 * NX :][:, cols]], axis=1)
                ).astype(bf16),
                "bqk_t": np.ascontiguousarray(bqk.reshape(8, P).T),
                "bv_r": np.ascontiguousarray(
                    c_attn_b[2 * NX :][cols].reshape(1, FEAT)
                ).astype(bf16),
                "wproj": np.ascontiguousarray(wproj_r).astype(bf16),
                "wpn3": np.ascontiguousarray(wpn3).astype(bf16),
                "bp_r": np.ascontiguousarray(
                    c_proj_b[cols].reshape(1, FEAT)
                ).astype(bf16),
            }
        )

    _LAST_IN_MAPS = in_maps
    if _NC_CACHE is None:
        _NC_CACHE = build()
    res = run_bass_kernel_spmd(_NC_CACHE, in_maps, core_ids=list(range(8)))
    outf = np.empty((B, S, NX), dtype=np.float32)
    for c in range(8):
        b, hg = c // 2, c % 2
        outf[b, :, hg * FEAT : (hg + 1) * FEAT] = res.results[c]["out"]
    return outf


# revision 27
# speedup vs baseline: 1.0764x; 1.0764x over previous
"""Distributed causal multi-head attention block (GPT-2 style) for 8 TRN2 NeuronCores.

Sharding: data-parallel over batch (4 groups of 2 cores) x tensor-parallel over
heads (2 groups of 8 heads). Core c handles batch c//2, head-group c%2.

v3: host pre-transposes/casts inputs to bf16 (xT, wqkv, wproj) and pre-shapes
the biases; the 16 input-chunk DMAs are first on the sync queue so the first
qkv wave (8 PSUM banks, emitted kc-major) tracks DMA arrival from ~2us and
warms the HAM clock gate early. Softmax normalization broadcasts the
reciprocal on GpSimd (partition_broadcast) instead of a PE rank-1 matmul,
removing a PE head-of-line stall per head. DMA traffic is split: input
chunks + gathered-aT reloads on the sync queue, biases + collective staging
on the Pool queue, so late-head staging never blocks early reloads.

Per-core pipeline (all matmuls bf16 with f32 PSUM accumulation):
  1. qT,kT = W^T chunks @ xT (feat-major), v = xT^T-chunks @ Wv (S-major)
  2. per head: scores^T tiles = kT_h^T-slices @ qT_h (causally skipped),
     P^T = exp(scores/8) (+ triangular mask on diagonal blocks),
     a[q,65] = P^T-blocks^T @ [v_h | ones]  -> denominator in col 64,
     normalize rows by 1/denom -> aT_loc bf16 [FEAT, S]
  3. pair AllGather of aT_loc chunks -> full a for the batch
  4. c_proj half-columns: out[q,512] = aT-chunks^T @ Wproj_half + bias
Host assembles out[b, :, hg*512:(hg+1)*512] from each core.
"""

import numpy as np
import ml_dtypes

import concourse.bass as bass
import concourse.mybir as mybir
import concourse.tile as tile
from concourse import bacc
from concourse.bass_utils import run_bass_kernel_spmd
from concourse.masks import make_upper_triangular

F32 = mybir.dt.float32
BF16 = mybir.dt.bfloat16
AF = mybir.ActivationFunctionType
ALU = mybir.AluOpType

P = 128
S = 1024          # sequence length
NX = 1024         # model width
D = 64            # head dim
H_LOC = 8         # heads per core
FEAT = H_LOC * D  # 512 local attention features
NKC = NX // P     # 8 contraction chunks
NST = S // P      # 8 sequence tiles
VW = D + 1        # v block width incl. ones column (65)


def build():
    nc = bacc.Bacc(num_devices=8)
    xT = nc.dram_tensor("xT", [NX, S], BF16, kind="ExternalInput")
    wqkv = nc.dram_tensor("wqkv", [NX, 3 * FEAT], BF16, kind="ExternalInput")
    bqk_t = nc.dram_tensor("bqk_t", [P, 8], F32, kind="ExternalInput")
    bv_r = nc.dram_tensor("bv_r", [1, FEAT], BF16, kind="ExternalInput")
    wproj = nc.dram_tensor("wproj", [NX, FEAT], BF16, kind="ExternalInput")
    bp_r = nc.dram_tensor("bp_r", [1, FEAT], BF16, kind="ExternalInput")
    out = nc.dram_tensor("out", [S, FEAT], F32, kind="ExternalOutput")

    with tile.TileContext(nc) as tc:
        with (
            tc.tile_pool(name="pt", bufs=16) as ptp,           # P^T blocks
            tc.tile_pool(name="small", bufs=4) as small,       # recip vectors
            tc.tile_pool(name="outp", bufs=3) as outp,         # out f32 tiles
            tc.tile_pool(name="dram", bufs=1, space="DRAM") as dram,
            tc.tile_pool(name="resident", bufs=1) as res,
        ):
            # ---- resident SBUF tensors (distinct tags -> distinct slots) ----
            xT_all = res.tile([P, NKC * S], BF16, tag="xT_all")          # [NX, S] chunked
            wqkv_sb = res.tile([P, NKC * 3 * FEAT], BF16, tag="wqkv_sb")
            qkT_all = res.tile([P, 8 * S], BF16, tag="qkT_all")          # qT(0..3)|kT(4..7)
            v_sb = res.tile([P, NST * H_LOC * VW], BF16, tag="v_sb")
            aT_loc = res.tile([P, 4 * S], BF16, tag="aT_loc")            # [FEAT, S] chunked
            wp_sb = res.tile([P, NKC * FEAT], BF16, tag="wp_sb")
            aT_all = res.tile([P, 16 * FEAT], BF16, tag="aT_all")        # (qh,fc) stage-3 lhsT
            bias_sb = res.tile([P, 8], F32, tag="bias_sb")
            bv_row = res.tile([1, FEAT], BF16, tag="bv_row")
            bp_row = res.tile([1, FEAT], BF16, tag="bp_row")
            ones_row = res.tile([1, P], BF16, tag="ones_row")
            utri = res.tile([P, P], BF16, tag="utri")

            nc.vector.memset(ones_row[:], 1.0)
            make_upper_triangular(nc, utri[:], val=1.0, diag=True)
            nc.vector.memset(v_sb[:], 1.0)

            # ---- input stream. sync queue: the 16 x/w chunks first (the
            # critical path), then wproj, then gathered-aT reloads, then the
            # qh=1 output tiles. Pool queue: tiny biases first, then per-head
            # broadcasts / collective staging in head order. ----
            for kc in range(NKC):
                xs = slice(kc * P, (kc + 1) * P)
                if kc < 2:
                    # split the first chunks into need-aligned pieces so the
                    # first qkv matmuls start as soon as ~200 KB has landed
                    nc.sync.dma_start(
                        wqkv_sb[:, kc * 3 * FEAT : kc * 3 * FEAT + 640],
                        wqkv[xs, 0:640],
                    )
                    nc.sync.dma_start(
                        xT_all[:, kc * S : kc * S + 512], xT[xs, 0:512]
                    )
                    nc.sync.dma_start(
                        xT_all[:, kc * S + 512 : (kc + 1) * S], xT[xs, 512:1024]
                    )
                    nc.sync.dma_start(
                        wqkv_sb[:, kc * 3 * FEAT + 640 : (kc + 1) * 3 * FEAT],
                        wqkv[xs, 640:1536],
                    )
                else:
                    nc.sync.dma_start(xT_all[:, kc * S : (kc + 1) * S], xT[xs, :])
                    nc.sync.dma_start(
                        wqkv_sb[:, kc * 3 * FEAT : (kc + 1) * 3 * FEAT],
                        wqkv[xs, :],
                    )
            nc.gpsimd.dma_start(bias_sb[:], bqk_t[:, :])
            nc.gpsimd.dma_start(bv_row[:], bv_r[:, :])
            nc.gpsimd.dma_start(bp_row[:], bp_r[:, :])
            for fc in range(NKC):
                nc.sync.dma_start(
                    wp_sb[:, fc * FEAT : (fc + 1) * FEAT],
                    wproj[fc * P : (fc + 1) * P, :],
                )

            # ---- qkv group helpers ----
            # qk group (ft, half): psum [128f, 512s]; ft 0..3 = q, 4..7 = k
            def qk_mm(ps, ft, half, kc):
                nc.tensor.matmul(
                    ps[:],
                    wqkv_sb[:, kc * 3 * FEAT + ft * P : kc * 3 * FEAT + (ft + 1) * P],
                    xT_all[:, kc * S + half * 512 : kc * S + (half + 1) * 512],
                    start=(kc == 0),
                    stop=(kc == NKC - 1),
                )

            def qk_consume(ps, ft, half):
                # bias-add + bf16 cast on DVE
                nc.vector.tensor_scalar_add(
                    out=qkT_all[:, ft * S + half * 512 : ft * S + (half + 1) * 512],
                    in0=ps[:],
                    scalar1=bias_sb[:, ft : ft + 1],
                )

            # v group (st): psum [128s, 512d]
            def v_bias(ps):
                nc.tensor.matmul(ps[:], ones_row[:, 0:P], bv_row[:], start=True, stop=False)

            def v_mm(ps, st, kc):
                nc.tensor.matmul(
                    ps[:],
                    xT_all[:, kc * S + st * P : kc * S + (st + 1) * P],
                    wqkv_sb[:, kc * 3 * FEAT + 1024 : kc * 3 * FEAT + 1536],
                    start=False,
                    stop=(kc == NKC - 1),
                )

            def v_consume(ps, st):
                # strided copy: 8 head blocks of 64 cols into the 65-wide
                # (v | ones) layout in one DVE instruction
                base = st * H_LOC * VW
                dst = v_sb[:, base : base + H_LOC * VW].rearrange(
                    "p (h w) -> p h w", h=H_LOC
                )[:, :, 0:D]
                src = ps[:].rearrange("p (h d) -> p h d", h=H_LOC)
                nc.vector.tensor_copy(out=dst, in_=src)

            # ---- gather plumbing (pair AllGather of aT_loc) ----
            PAIRS = [[0, 1], [2, 3], [4, 5], [6, 7]]
            cc_in0 = dram.tile([FEAT, 512], BF16, name="cc_in0")
            cc_out0 = dram.tile([2 * FEAT, 512], BF16, name="cc_out0")
            FT_PARTS = [[0, 1], [2], [3]]
            cc_in1 = [
                dram.tile([len(fts) * P, 512], BF16, name=f"cc_in1{i}")
                for i, fts in enumerate(FT_PARTS)
            ]
            cc_out1 = [
                dram.tile([2 * len(fts) * P, 512], BF16, name=f"cc_out1{i}")
                for i, fts in enumerate(FT_PARTS)
            ]

            def gather_half0():
                for ft in range(4):
                    nc.gpsimd.dma_start(
                        cc_in0[ft * P : (ft + 1) * P, :],
                        aT_loc[:, ft * S : ft * S + 512],
                    )
                nc.gpsimd.collective_compute(
                    "AllGather", ALU.bypass, replica_groups=PAIRS,
                    ins=[cc_in0[:].opt()], outs=[cc_out0[:].opt()],
                )

            def gather_half1(part):
                for i, ft in enumerate(FT_PARTS[part]):
                    nc.gpsimd.dma_start(
                        cc_in1[part][i * P : (i + 1) * P, :],
                        aT_loc[:, ft * S + 512 : (ft + 1) * S],
                    )
                nc.gpsimd.collective_compute(
                    "AllGather", ALU.bypass, replica_groups=PAIRS,
                    ins=[cc_in1[part][:].opt()], outs=[cc_out1[part][:].opt()],
                )

            def _gathered_src(qh2, fc):
                # global feature chunk fc: rank block fc//4, local ft fc%4
                blk, lft = fc // 4, fc % 4
                if qh2 == 0:
                    return cc_out0[(blk * 4 + lft) * P : (blk * 4 + lft + 1) * P, :]
                part = 0 if lft < 2 else lft - 1
                i = lft if lft < 2 else 0
                n = len(FT_PARTS[part])
                return cc_out1[part][(blk * n + i) * P : (blk * n + i + 1) * P, :]

            def proj_load(qh2, fcs=None):
                # reload gathered aT on the sync queue (nothing early sits
                # behind these; staging DMAs live on the Pool queue)
                for fc in fcs or range(NKC):
                    nc.sync.dma_start(
                        aT_all[:, (qh2 * NKC + fc) * FEAT : (qh2 * NKC + fc + 1) * FEAT],
                        _gathered_src(qh2, fc),
                    )

            # ---- wave 1: 8 groups fed in DMA-arrival order (kc-major) ----
            # groups: qk(0,0), qk(0,1), qk(4,0), qk(4,1), v(0..3) — exactly the
            # inputs heads 0/1 of q-half 0 need first
            W1_QK = [(0, 0), (0, 1), (4, 0), (4, 1)]
            W1_V = [0, 1, 2, 3]
            with tc.tile_pool(name="ps_w1", bufs=8, space="PSUM") as psw:
                w1ps = {}
                for ft, half in W1_QK:
                    w1ps[("qk", ft, half)] = psw.tile(
                        [P, 512], F32, name=f"w1qk{ft}{half}", tag="w1"
                    )
                for st in W1_V:
                    ps = psw.tile([P, 512], F32, name=f"w1v{st}", tag="w1")
                    w1ps[("v", st)] = ps
                    v_bias(ps)
                for kc in range(NKC):
                    # halves adjacent: q(0) then k(4) match first-chunk pieces
                    for ft, half in [(0, 0), (4, 0), (0, 1), (4, 1)]:
                        qk_mm(w1ps[("qk", ft, half)], ft, half, kc)
                    for st in W1_V:
                        v_mm(w1ps[("v", st)], st, kc)
                for ft, half in W1_QK:
                    qk_consume(w1ps[("qk", ft, half)], ft, half)
                for st in W1_V:
                    v_consume(w1ps[("v", st)], st)

            # ---- attention phase (with remaining qkv tiles interleaved) ----
            with (
                tc.tile_pool(name="ps_big", bufs=2, space="PSUM") as ps_big,
                tc.tile_pool(name="ps_sc", bufs=3, space="PSUM") as ps_sc,
            ):
                # later qkv groups: SBUF-fed, rotate through ps_big
                def qkT_tile(ft):
                    for half in range(2):
                        ps = ps_big.tile([P, 512], F32, name="ps_qk", tag="big")
                        for kc in range(NKC):
                            qk_mm(ps, ft, half, kc)
                        qk_consume(ps, ft, half)

                def v_tile(st):
                    ps = ps_big.tile([P, 512], F32, name="ps_v", tag="big")
                    v_bias(ps)
                    for kc in range(NKC):
                        v_mm(ps, st, kc)
                    v_consume(ps, st)

                def attention_head(qh, h):
                    nj = 4 * qh + 4                    # causal k-tiles for this half
                    prow = (h % 2) * D
                    qcol = (h // 2) * S            # qT feature-tile col base
                    kcol = (4 + h // 2) * S        # kT feature-tile col base
                    pt_blocks = []
                    for j in range(nj):
                        dloc = j - 4 * qh          # diagonal block index in this half
                        coff = max(dloc, 0) * P    # first allowed local q col
                        ps = ps_sc.tile([P, 512], F32)
                        ptb = ptp.tile([P, 512], BF16, tag="pt")
                        nc.tensor.matmul(
                            ps[:, coff:512],
                            qkT_all[prow : prow + D, kcol + j * P : kcol + (j + 1) * P],
                            qkT_all[
                                prow : prow + D,
                                qcol + qh * 512 + coff : qcol + (qh + 1) * 512,
                            ],
                            start=True,
                            stop=True,
                        )
                        nc.scalar.activation(
                            out=ptb[:, coff:512],
                            in_=ps[:, coff:512],
                            func=AF.Exp,
                            scale=0.125,
                        )
                        if dloc >= 0:
                            nc.vector.tensor_tensor(
                                out=ptb[:, coff : coff + P],
                                in0=ptb[:, coff : coff + P],
                                in1=utri[:],
                                op=ALU.mult,
                            )
                        pt_blocks.append((ptb, coff))
                    # aT[d, q] for this (head, half) + denominator row via ones
                    # col; each k-block only contributes to its causal q cols
                    psa = ps_sc.tile([VW, 512], F32, tag="psaT", bufs=3)
                    for j in range(nj):
                        ptb, coff = pt_blocks[j]
                        nc.tensor.matmul(
                            psa[:, coff:512],
                            v_sb[:, j * H_LOC * VW + h * VW : j * H_LOC * VW + (h + 1) * VW],
                            ptb[:, coff:512],
                            start=(j == 0),
                            stop=(j == nj - 1),
                        )
                    # fast-recip the denominator row, broadcast it down 64
                    # partitions on GpSimd, normalize straight out of PSUM
                    acols = slice((h // 2) * S + qh * 512, (h // 2) * S + (qh + 1) * 512)
                    db = small.tile([1, 512], F32, tag="db")
                    nc.vector.tensor_copy(out=db[:], in_=psa[D : D + 1, :])
                    rc = small.tile([1, 512], F32, tag="rc")
                    nc.vector.reciprocal_approx_fast(rc[:], db[:])
                    bcs = small.tile([D, 512], F32, tag="bcs")
                    nc.gpsimd.partition_broadcast(bcs[:], rc[:])
                    nc.vector.tensor_tensor(
                        out=aT_loc[prow : prow + D, acols],
                        in0=bcs[:],
                        in1=psa[0:D, :],
                        op=ALU.mult,
                    )

                # ---- interleaved emission: weave remaining qkv tiles between
                # attention heads so exp (ACT) spreads and PE never starves ----
                qkT_tile(1)
                qkT_tile(5)
                attention_head(0, 0)
                attention_head(0, 1)
                qkT_tile(2)
                qkT_tile(6)
                attention_head(0, 2)
                attention_head(0, 3)
                qkT_tile(3)
                qkT_tile(7)
                attention_head(0, 4)
                attention_head(0, 5)
                v_tile(4)
                v_tile(5)
                attention_head(0, 6)
                attention_head(0, 7)
                gather_half0()
                v_tile(6)
                v_tile(7)
                for h in range(4):
                    attention_head(1, h)
                gather_half1(0)  # ft0/ft1 of qh=1 fly while heads 4-7 compute
                proj_load(0)     # AG#0 result; loads overlap remaining attention
                attention_head(1, 4)
                attention_head(1, 5)
                gather_half1(1)  # ft2 flies while heads 6-7 compute
                proj_load(1, [0, 1, 4, 5])  # prefetch from the early gather
                attention_head(1, 6)
                proj_load(1, [2, 6])        # lands as soon as gather#2 does
                attention_head(1, 7)
                gather_half1(2)

            # ---- c_proj: attention PSUM pools are closed, use a wide pool.
            # Pre-accumulate early-gathered chunks for proj(1); proj(0) tiles
            # keep the PE warm through the final gather's wait; the LATE
            # chunks finish once the last 128 KB gather lands ----
            with tc.tile_pool(name="ps_pj", bufs=6, space="PSUM") as ps_pj:

                def proj_acc(qh2, lt, ps, fcs, first, last):
                    if first:
                        nc.tensor.matmul(
                            ps[:], ones_row[:, 0:P], bp_row[:], start=True, stop=False
                        )
                    for n, fc in enumerate(fcs):
                        nc.tensor.matmul(
                            ps[:],
                            aT_all[
                                :,
                                (qh2 * NKC + fc) * FEAT + lt * P
                                : (qh2 * NKC + fc) * FEAT + (lt + 1) * P,
                            ],
                            wp_sb[:, fc * FEAT : (fc + 1) * FEAT],
                            start=False,
                            stop=(last and n == len(fcs) - 1),
                        )
                    if last:
                        t = 4 * qh2 + lt
                        ot = outp.tile([P, FEAT], F32, tag="ot")
                        if qh2 == 1 and lt % 2 == 0:
                            nc.vector.tensor_copy(out=ot[:], in_=ps[:])
                            nc.sync.dma_start(out[t * P : (t + 1) * P, :], ot[:])
                        elif qh2 == 1:
                            nc.scalar.copy(ot[:], ps[:])  # ACT is idle at the tail
                            nc.sync.dma_start(out[t * P : (t + 1) * P, :], ot[:])
                        else:
                            nc.scalar.copy(ot[:], ps[:])
                            nc.gpsimd.dma_start(out[t * P : (t + 1) * P, :], ot[:])

                def proj_tile(qh2, lt):
                    ps = ps_pj.tile([P, 512], F32, name="ps_pj0", tag="pj")
                    proj_acc(qh2, lt, ps, list(range(NKC)), True, True)

                EARLY, LATE = [0, 1, 4, 5, 2, 6], [3, 7]
                ps1 = {}
                for lt in range(3):
                    ps1[lt] = ps_pj.tile([P, 512], F32, name=f"ps_p1{lt}", tag="pj")
                    proj_acc(1, lt, ps1[lt], EARLY, True, False)
                for lt in range(4):
                    proj_tile(0, lt)  # fills the final gather's wait
                proj_load(1, [3, 7])
                ps1[3] = ps_pj.tile([P, 512], F32, name="ps_p13", tag="pj")
                proj_acc(1, 3, ps1[3], EARLY, True, False)
                proj_acc(1, 0, ps1[0], LATE, False, True)
                proj_acc(1, 1, ps1[1], LATE, False, True)
                proj_acc(1, 2, ps1[2], LATE, False, True)
                proj_acc(1, 3, ps1[3], LATE, False, True)

    nc.finalize()
    return nc


_NC_CACHE = None
_LAST_IN_MAPS = None


def kernel(x, c_attn_w, c_attn_b, c_proj_w, c_proj_b):
    global _NC_CACHE, _LAST_IN_MAPS
    x = np.asarray(x, dtype=np.float32)
    c_attn_w = np.asarray(c_attn_w, dtype=np.float32)
    c_attn_b = np.asarray(c_attn_b, dtype=np.float32)
    c_proj_w = np.asarray(c_proj_w, dtype=np.float32)
    c_proj_b = np.asarray(c_proj_b, dtype=np.float32)
    B = x.shape[0]
    assert x.shape == (B, S, NX)
    bf16 = ml_dtypes.bfloat16

    xTs = [np.ascontiguousarray(x[b].T).astype(bf16) for b in range(B)]
    in_maps = []
    for c in range(8):
        b, hg = c // 2, c % 2
        cols = slice(hg * FEAT, (hg + 1) * FEAT)
        wq = c_attn_w[:, 0 * NX :][:, cols]
        wk = c_attn_w[:, 1 * NX :][:, cols]
        wv = c_attn_w[:, 2 * NX :][:, cols]
        bq = c_attn_b[0 * NX :][cols]
        bk = c_attn_b[1 * NX :][cols]
        bqk = np.concatenate([bq, bk])                       # [1024]
        in_maps.append(
            {
                "xT": xTs[b],
                "wqkv": np.ascontiguousarray(
                    np.concatenate([wq, wk, c_attn_w[:, 2 * NX :][:, cols]], axis=1)
                ).astype(bf16),
                "bqk_t": np.ascontiguousarray(bqk.reshape(8, P).T),
                "bv_r": np.ascontiguousarray(
                    c_attn_b[2 * NX :][cols].reshape(1, FEAT)
                ).astype(bf16),
                "wproj": np.ascontiguousarray(c_proj_w[:, cols]).astype(bf16),
                "bp_r": np.ascontiguousarray(
                    c_proj_b[cols].reshape(1, FEAT)
                ).astype(bf16),
            }
        )

    _LAST_IN_MAPS = in_maps
    if _NC_CACHE is None:
        _NC_CACHE = build()
    res = run_bass_kernel_spmd(_NC_CACHE, in_maps, core_ids=list(range(8)))
    outf = np.empty((B, S, NX), dtype=np.float32)
    for c in range(8):
        b, hg = c // 2, c % 2
        outf[b, :, hg * FEAT : (hg + 1) * FEAT] = res.results[c]["out"]
    return outf

